# revision 1
# baseline (speedup 1.0000x reference)
"""Trainium2 Bass kernel: batched cosine-similarity relation matrix.

Computes out[b,i,j,m,n] = <q_hat[b,i,m,:], s_hat[b,j,n,:]> where q_hat/s_hat
are L2-normalized along k (torch F.normalize semantics, eps=1e-12).

Shapes (hardcoded): query/support [4, 25, 128, 64] f32 -> out [4, 25, 25, 128, 128] f32.

Sharding: 8 cores = (b, i-half) grid. Core c handles b=c//2 and i-rows
[13*h, 13*h+13) with i padded 25->26 (h=c%2). Each core computes its
[13, 25, 128, 128] slice independently; no communication.

Device pipeline per core (inputs arrive pre-transposed, contraction dim k on
partitions -- a pure host-side layout change):
  1. SWDGE-load qT [64, 13*128] / sT [64, 25*128] fp32 in 512-col chunks.
  2. Per chunk: square (DVE), ones-matmul (PE) -> sum_k sq replicated across
     psum partitions, sqrt(+1e-24) (ACT), reciprocal (DVE), then one
     multiply-and-cast to fp16 (DVE) -> L2-normalized fp16 operands.
  3. 13 x 7 matmuls: psum[128m, 512] = qT16[64,128].T @ sT16[64, 4*128]
     (fp16 in, fp32 accumulate).
  4. PSUM->SBUF copies (split ACT/DVE), DMA out: small per-block DMAs for the
     first row (fast ramp), then one 1.6MB DMA per row (amortizes per-DMA
     HWDGE overhead).
"""

import os

import numpy as np

import concourse.bacc as bacc
import concourse.bass as bass
import concourse.mybir as mybir
import concourse.tile as tile
from concourse.bass_utils import run_bass_kernel_spmd

B, I, M, K = 4, 25, 128, 64
J, N = 25, 128
II = 13  # i-rows per core (i padded to 26 = 2 halves of 13)
NCORES = 8

# Stash of the most recent BassKernelResults (test.py reads exec_time_ns).
last_results = None

_nc_cache = {}


def _build_nc(
    mm_dtype=mybir.dt.float16,
    ob_bufs=4,
    mm_bufs=6,
    copy_pattern="aav",
    jblk=4,
    ramp_rows=2,
    reps=1,
    bench_tag=0,
    dbg_no_out_dma=False,
):
    f32 = mybir.dt.float32
    nc = bacc.Bacc(trn_type="TRN2")
    qT_d = nc.dram_tensor("qT", [K, II * M], f32, kind="ExternalInput")
    sT_d = nc.dram_tensor("sT", [K, J * N], f32, kind="ExternalInput")
    out = nc.dram_tensor("out", [II, J, M, N], f32, kind="ExternalOutput")
    if bench_tag:
        # Bench-only: extra dummy input of a distinctive size so the jitted
        # HLO (and thus the neuron compile-cache key) differs per variant --
        # the cache key ignores the embedded BIR.
        pad_d = nc.dram_tensor("pad", [1, bench_tag], f32, kind="ExternalInput")

    jblocks = [(j0, min(jblk, J - j0)) for j0 in range(0, J, jblk)]
    # First-row blocks are narrow so the first output DMA fires ~2.5us in
    # (each narrow block has a short normalize chain ahead of it).
    jblocks_ramp = [(0, 1), (1, 1), (2, 2), (4, 4), (8, 4), (12, 4), (16, 4), (20, 4), (24, 1)]
    psum_w = jblk * N
    assert psum_w <= 512

    with tile.TileContext(nc) as tc:
        with (
            tc.tile_pool(name="const", bufs=1) as const,
            tc.tile_pool(name="inp", bufs=1) as inp,
            tc.tile_pool(name="mmp", bufs=mm_bufs, space="PSUM") as mmp,
            tc.tile_pool(name="npp", bufs=2, space="PSUM") as npp,
            tc.tile_pool(name="obp", bufs=ob_bufs) as obp,
        ):
            eps_t = const.tile([128, 1], f32)
            nc.vector.memset(eps_t, 1e-24)
            ones_t = const.tile([K, K], f32)
            nc.vector.memset(ones_t, 1.0)
            # Dummy Sqrt up front: absorbs the ACT table switch to
            # "sqrt_and_others" (covers sqrt/copy) on an instruction with few
            # waits -- walrus rejects table-load waits piled onto an
            # instruction that already has multiple sem waits.
            warm = const.tile([128, 1], f32)
            nc.scalar.activation(
                out=warm,
                in_=eps_t,
                func=mybir.ActivationFunctionType.Sqrt,
                bias=eps_t,
            )

            if bench_tag:
                pad_sb = const.tile([1, bench_tag], f32)
                nc.gpsimd.dma_start(out=pad_sb, in_=pad_d[:])

            qT_raw = inp.tile([K, II, M], f32)
            sT_raw = inp.tile([K, J, N], f32)
            qT16 = inp.tile([K, II, M], mm_dtype)
            sT16 = inp.tile([K, J, N], mm_dtype)

            def _body():
                # 4 input DMAs total. The two small HEAD loads go through
                # HWDGE (idle at t=0, ~0.6us overhead) so the first chunk
                # chains start ~1.5us earlier; the big tail loads go through
                # SWDGE (Pool) to stay off the output HWDGE ring.
                def load_input(x_dram, x_raw, a, w, width, eng):
                    eng.dma_start(
                        out=x_raw[:, a : a + w, :],
                        in_=x_dram[:, a * width : (a + w) * width].rearrange(
                            "k (t c) -> k t c", t=w
                        ),
                    )

                load_input(qT_d, qT_raw, 0, 1, M, nc.sync)
                load_input(sT_d, sT_raw, 0, 4, N, nc.sync)
                load_input(sT_d, sT_raw, 4, J - 4, N, nc.gpsimd)
                load_input(qT_d, qT_raw, 1, II - 1, M, nc.gpsimd)

                # Stats chunks: narrow early (short chains), wide later.
                q_chunks = [(0, 1), (1, 4), (5, 4), (9, 4)]
                s_chunks = [(0, 1), (1, 1), (2, 2), (4, 4), (8, 4), (12, 4), (16, 4), (20, 4), (24, 1)]

                def prep_chunk(x_raw, x16, a, w, width):
                    """Normalize chunk [64, w*width] along k, cast to fp16."""
                    xs = x_raw[:, a : a + w, :]
                    fw = w * width
                    sq_c = inp.tile([K, 512], f32, tag="sq", name="sq_c", bufs=3)
                    # Square on GpSimd: the Pool engine is otherwise idle, and this
                    # halves the serial DVE prep chain.
                    nc.gpsimd.tensor_mul(sq_c[:, :fw], xs, xs)
                    # ones[64,64].T @ sq[64, fw] -> psum[64, fw] where every
                    # partition row holds sum_k sq[k, c] = ||x_c||^2.
                    np_t = npp.tile([K, 512], f32, tag="np", name="np_t")
                    nc.tensor.matmul(
                        np_t[:, :fw],
                        lhsT=ones_t,
                        rhs=sq_c[:, :fw],
                        start=True,
                        stop=True,
                    )
                    inv_c = inp.tile([K, 512], f32, tag="inv", name="inv_c", bufs=3)
                    # sqrt(sumsq + 1e-24): zero (padded) rows -> norm 1e-12,
                    # matching the reference's max(norm, 1e-12), no inf/nan.
                    nc.scalar.activation(
                        out=inv_c[:, :fw],
                        in_=np_t[:, :fw],
                        func=mybir.ActivationFunctionType.Sqrt,
                        bias=eps_t[:K],
                    )
                    nc.vector.reciprocal(out=inv_c[:, :fw], in_=inv_c[:, :fw])
                    x16s = x16[:, a : a + w, :].rearrange("k t c -> k (t c)")
                    nc.vector.tensor_mul(x16s, xs.rearrange("k t c -> k (t c)"), inv_c[:, :fw])

                q_done = [False] * len(q_chunks)
                s_done = [False] * len(s_chunks)

                def ensure_q(ii):
                    for c, (a, w) in enumerate(q_chunks):
                        if a <= ii < a + w and not q_done[c]:
                            prep_chunk(qT_raw, qT16, a, w, M)
                            q_done[c] = True

                def ensure_s(j_hi):
                    for c, (a, w) in enumerate(s_chunks):
                        if a < j_hi and not s_done[c]:
                            prep_chunk(sT_raw, sT16, a, w, N)
                            s_done[c] = True

                it = 0
                for ii in range(II):
                    ensure_q(ii)
                    row_big = ii >= ramp_rows
                    if row_big:
                        big = obp.tile([M, J, N], f32, tag="ob", name="big")
                    row_blocks = jblocks_ramp if ii == 0 else jblocks
                    for j0, w in row_blocks:
                        if ii == 0:
                            ensure_s(j0 + w)
                        wn = w * N
                        ps = mmp.tile([M, psum_w], f32, tag="mm", name="ps")
                        nc.tensor.matmul(
                            ps[:, :wn],
                            lhsT=qT16[:, ii, :],
                            rhs=sT16[:, j0 : j0 + w, :],
                            start=True,
                            stop=True,
                        )
                        if row_big:
                            o_t = big[:, j0 : j0 + w, :].rearrange("m j n -> m (j n)")
                        else:
                            o_tile = obp.tile(
                                [M, psum_w], f32, tag="obs", name="o_tile", bufs=8
                            )
                            o_t = o_tile[:, :wn]
                        # Split PSUM->SBUF copies between ACT ('a') and DVE ('v').
                        if copy_pattern == "smart":
                            eng = "a" if ii < ramp_rows else ("v" if ii < ramp_rows + 2 else "aav"[it % 3])
                        else:
                            eng = copy_pattern[it % len(copy_pattern)]
                        if eng == "a":
                            nc.scalar.copy(out=o_t, in_=ps[:, :wn])
                        else:
                            nc.vector.tensor_copy(out=o_t, in_=ps[:, :wn])
                        if not row_big and not dbg_no_out_dma:
                            # Ramp rows: small per-block DMAs so the output stream
                            # starts as soon as the first block is ready.
                            nc.sync.dma_start(
                                out=out[ii, j0 : j0 + w].rearrange("j m n -> m j n"),
                                in_=o_t.rearrange("m (j n) -> m j n", j=w),
                            )
                        it += 1
                    if row_big and not dbg_no_out_dma:
                        # Steady state: one 1.6MB DMA per i-row amortizes the
                        # per-DMA HWDGE overhead (~0.6us).
                        nc.sync.dma_start(
                            out=out[ii].rearrange("j m n -> m j n"),
                            in_=big,
                        )

            if reps > 1:
                # Benchmark mode: repeat the whole pipeline in a HW loop so
                # per-call tunnel overhead can be cancelled by slope fitting.
                with tc.For_i(0, reps, 1):
                    _body()
            else:
                _body()
    nc.compile()
    return nc


def _get_nc():
    if "nc" not in _nc_cache:
        _nc_cache["nc"] = _build_nc()
    return _nc_cache["nc"]


def _shard_inputs(query, support):
    q = np.ascontiguousarray(np.asarray(query, dtype=np.float32))
    s = np.ascontiguousarray(np.asarray(support, dtype=np.float32))
    qpad = np.zeros((B, 2 * II, M, K), dtype=np.float32)
    qpad[:, :I] = q
    in_maps = []
    for c in range(NCORES):
        b, h = divmod(c, 2)
        # [II, M, K] -> [K, II*M]: contraction dim on partitions, so the
        # device needs no transposes (pure host-side relayout).
        qc = np.ascontiguousarray(
            qpad[b, h * II : (h + 1) * II].transpose(2, 0, 1).reshape(K, II * M)
        )
        sc = np.ascontiguousarray(s[b].transpose(2, 0, 1).reshape(K, J * N))
        in_maps.append({"qT": qc, "sT": sc})
    return in_maps


def kernel(query, support):
    global last_results
    nc = _get_nc()
    in_maps = _shard_inputs(query, support)
    trace = bool(int(os.environ.get("BASS_KERNEL_TRACE", "0")))
    if not trace:
        # The axon client here has no NTFF hook; an external BASS_TRACE=1
        # would crash run_bass_kernel_spmd on a missing import.
        os.environ.setdefault("BASS_NEVER_TRACE", "1")
    res = run_bass_kernel_spmd(
        nc,
        in_maps,
        core_ids=list(range(NCORES)),
        trace=trace,
    )
    last_results = res
    full = np.empty((B, I, J, M, N), dtype=np.float32)
    for c in range(NCORES):
        b, h = divmod(c, 2)
        i0 = h * II
        i1 = min(i0 + II, I)
        full[b, i0:i1] = res.results[c]["out"][: i1 - i0]
    return full

